# revision 40
# baseline (speedup 1.0000x reference)
"""Trainium2 Bass kernel for nn_AttentionModel_88905823027207 (v5).

Full inputs:  x [4, 2048, 1024] f32, w_qkv [1024, 3072] f32, w_out [1024, 1024] f32
Full output:  [4, 2048, 1024] f32  (multi-head attention, 16 heads, + out proj)

Sharding: 8 cores = (batch b in 0..3) x (head-group g in 0..1).
Each core computes 8 heads of one batch element and the partial out-projection
for its head-group's rows of w_out; the host sums the two partials per batch.

v5 structure: the kernel streams 256 attention iterations, each gated by one
~1.11us [128,1024] exp ACTIVATE (the ACT roofline, ~285us total).  All bf16:
fp8 was measured to be numerically unusable here -- attention output is a
weighted MEAN of v, so the signal shrinks exactly as fast as averaged
quantization noise and every fp8 tensor (x/w/v/exp) contributes its full
~1.8-4% elementwise error to the output (measured 6.5% total vs the 2e-2
budget).  Relative to the v2 baseline the wins are pure scheduling:
  - stage-1 (qkv projections) and the out-projection run as due-date fillers
    inside the stream instead of a 76us solo-PE prefix and a 36us tail;
  - redundant scores matmuls pad PE-idle slots so the HAM clock gate never
    sees an idle MID window (at <70% busy the PE is held at 1.2 GHz);
  - input DMAs are ordered so iteration 0 unblocks after two transfers;
  - denominator reciprocals run 64 lanes wide ([8,512]->[64,64] reshape) via
    the fast approximate reciprocal (~8x+5x cheaper);
  - the final normalize chain DMA-shifts a_sb halves and broadcasts recips
    across all 128 partitions so one mul writes attn_norm directly, and the
    last pass copies its denominator row out of PSUM before the big
    evacuation; tail evacuations alternate between the (idle) ACT and DVE.
"""

import numpy as np
import ml_dtypes

BF16 = ml_dtypes.bfloat16

# Full-problem dims (hardcoded per harness contract)
B_FULL, S_FULL, D_FULL, H_FULL, HD = 4, 2048, 1024, 16, 64
N_CORES = 8
HEADS_PER_CORE = H_FULL // 2  # 8


def build_nc(S=2048, D=1024, heads=8, debug=False, do_compile=True):
    """Build + compile the per-core Bass program."""
    import concourse.bass as bass
    import concourse.mybir as mybir
    import concourse.tile as tile
    from concourse import bacc

    f32 = mybir.dt.float32
    bf16 = mybir.dt.bfloat16
    FT = mybir.ActivationFunctionType

    E = heads * HD              # per-core head channels (512)
    NDT = D // 128              # d-tiles (8)
    NST = S // 128              # s-tiles / k-tiles (16)
    NSC = S // 512              # 512-wide s-chunks (4)
    NET = E // 128              # e-tiles == head pairs (4)
    NQC = S // 512              # q-chunks (4)
    VW = 65                     # v columns per head incl. ones column
    NIT = NQC * NET * NST       # 256 attention iterations

    nc = bacc.Bacc("TRN2", target_bir_lowering=False, debug=debug)

    # inputs are host-prearranged to [partition, tile, free] so every input
    # DMA is a regular low-descriptor pattern (strided-gather issue cost on
    # the sync queue was ~1-3us per DMA and serialized the whole startup)
    xT_d = nc.dram_tensor("xT", [128, NDT, S], bf16, kind="ExternalInput")
    wq_d = nc.dram_tensor("wq", [128, NDT, E], bf16, kind="ExternalInput")
    wk_d = nc.dram_tensor("wk", [128, NDT, E], bf16, kind="ExternalInput")
    wv_d = nc.dram_tensor("wv", [128, NDT, E], bf16, kind="ExternalInput")
    wo_d = nc.dram_tensor("wo", [128, NET, D], bf16, kind="ExternalInput")
    out_d = nc.dram_tensor("out", [S, D], f32, kind="ExternalOutput")

    from contextlib import ExitStack

    with tile.TileContext(nc) as tc, ExitStack() as ctx:
        const = ctx.enter_context(tc.tile_pool(name="const", bufs=1))
        proj_ps = ctx.enter_context(tc.tile_pool(name="proj_ps", bufs=2, space="PSUM"))
        scores_ps = ctx.enter_context(tc.tile_pool(name="scores_ps", bufs=2, space="PSUM"))
        attn_ps = ctx.enter_context(tc.tile_pool(name="attn_ps", bufs=1, space="PSUM"))
        # deep exp pool: attnV may lag up to 9 iterations behind the ACT
        # stream, letting the PE absorb the front-loaded stage-1 burst
        # (vhat/kT deadlines) without stalling the ACT engine
        expp = ctx.enter_context(tc.tile_pool(name="expp", bufs=8))
        asbp = ctx.enter_context(tc.tile_pool(name="asbp", bufs=5))
        rcp = ctx.enter_context(tc.tile_pool(name="rcp", bufs=2))
        bcastp = ctx.enter_context(tc.tile_pool(name="bcastp", bufs=4))
        outst = ctx.enter_context(tc.tile_pool(name="outst", bufs=3))
        dramp = ctx.enter_context(tc.tile_pool(name="dramp", bufs=4, space="DRAM"))

        # ---- persistent SBUF tensors ----
        xT_sb = const.tile([128, NDT, S], bf16, tag="xT_sb")
        wq_sb = const.tile([128, NDT, E], bf16, tag="wq_sb")
        wk_sb = const.tile([128, NDT, E], bf16, tag="wk_sb")
        wv_sb = const.tile([128, NDT, E], bf16, tag="wv_sb")
        wo_sb = const.tile([128, NET, D], bf16, tag="wo_sb")
        qT = [const.tile([128, S], bf16, tag=f"qT{p}", name=f"qT{p}") for p in range(NET)]
        kT = [const.tile([128, S], bf16, tag=f"kT{p}", name=f"kT{p}") for p in range(NET)]
        vhat = [const.tile([128, heads, VW], bf16, tag=f"vh{st}", name=f"vh{st}") for st in range(NST)]
        attn_norm = [const.tile([128, S], bf16, tag=f"an{p}", name=f"an{p}") for p in range(NET)]
        # denominators gathered as [64,64] (8 rows of 64 per head) so the
        # reciprocal runs 64 lanes wide instead of 8
        den = [const.tile([64, 64], f32, tag=f"den{qc}", name=f"den{qc}") for qc in range(NQC - 1)]

        # ---- input DMAs in first-use order (whole tensors: per-slice DMAs
        # measured slower -- issue serialization beats the pipelining win) ----
        nc.sync.dma_start(out=wk_sb, in_=wk_d.ap())
        for c in range(NSC):
            sl = slice(c * 512, (c + 1) * 512)
            nc.sync.dma_start(out=xT_sb[:, :, sl], in_=xT_d.ap()[:, :, sl])
            if c == 0:
                nc.sync.dma_start(out=wq_sb, in_=wq_d.ap())
                nc.sync.dma_start(out=wv_sb, in_=wv_d.ap())
        nc.sync.dma_start(out=wo_sb, in_=wo_d.ap())

        # ---- emit helpers (each emits one atomic filler unit) ----
        def emit_vhat(st):
            # vhat[st][:, :, 0:64] = x[st-tile] @ wv (all heads, N=512)
            ps = proj_ps.tile([128, 512], f32, tag="proj")
            for dt in range(NDT):
                nc.tensor.matmul(
                    ps,
                    lhsT=xT_sb[:, dt, st * 128:(st + 1) * 128],
                    rhs=wv_sb[:, dt, :],
                    start=(dt == 0),
                    stop=(dt == NDT - 1),
                )
            nc.vector.tensor_copy(
                out=vhat[st][:, :, 0:HD],
                in_=ps.rearrange("q (h c) -> q h c", c=HD),
            )

        def emit_qkT(w_sb, dstT, p, c):
            # dstT[p][:, chunk c] = w[:, pair p].T @ x[:, chunk c] (N=512)
            ps = proj_ps.tile([128, 512], f32, tag="proj")
            for dt in range(NDT):
                nc.tensor.matmul(
                    ps,
                    lhsT=w_sb[:, dt, p * 128:(p + 1) * 128],
                    rhs=xT_sb[:, dt, c * 512:(c + 1) * 512],
                    start=(dt == 0),
                    stop=(dt == NDT - 1),
                )
            nc.vector.tensor_copy(out=dstT[p][:, c * 512:(c + 1) * 512], in_=ps)

        def outproj_mms(ps, st, dc, plist):
            for p in plist:
                nc.tensor.matmul(
                    ps,
                    lhsT=attn_norm[p][:, st * 128:(st + 1) * 128],
                    rhs=wo_sb[:, p, dc * 512:(dc + 1) * 512],
                    start=(p == 0),
                    stop=(p == NET - 1),
                )

        def outproj_finish(ps, st, dc, engine=None):
            ot = outst.tile([128, 512], f32, tag="ot")
            out_ap = out_d.ap()[st * 128:(st + 1) * 128, dc * 512:(dc + 1) * 512]
            if engine == "scalar":
                # tail only: ACT is idle after the last exp; splitting the
                # evacuations and the final stores across the two hwdge
                # queues halves the end-of-kernel drain
                nc.scalar.copy(out=ot, in_=ps)
                nc.scalar.dma_start(out=out_ap, in_=ot)
            else:
                nc.vector.tensor_copy(out=ot, in_=ps)
                nc.sync.dma_start(out=out_ap, in_=ot)

        def emit_outproj(st, dc):
            ps = proj_ps.tile([128, 512], f32, tag="proj")
            outproj_mms(ps, st, dc, range(NET))
            outproj_finish(ps, st, dc)

        # ---- prefix: first vhat memsets (ones columns) on the idle GpSimd
        # engine (keeps the DVE free for the first evacuations; the rest are
        # staggered fillers) + the two stage-1 units gating iteration 0's
        # scores: kT[0] c0, qT[0] c0.  vhat kt0/kt1 run as due<0 fillers
        # inside the stream, under ACT(0).
        nc.gpsimd.memset(vhat[0], 1.0)
        nc.gpsimd.memset(vhat[1], 1.0)
        # constants for the tail's PE-broadcast normalize
        ones_t = const.tile([1, 128], f32, tag="ones_t")
        nc.gpsimd.memset(ones_t, 1.0)
        # PE warmup: the HAM clock gate releases 2.4 GHz only after ~3.4us of
        # sustained activity; a handful of dummy matmuls during the DMA wait
        # means kT[0]c0/qT[0]c0 and the early stream run warm
        warm_t = const.tile([128, 640], bf16, tag="warm_t")
        nc.gpsimd.memset(warm_t, 0.0)
        # ~28 dummies span the ~10us input-DMA wait: HAM warms ~3.4us in and
        # kT[0]c0 plus the early stream then run at 2.4 GHz from the start
        wps = proj_ps.tile([128, 512], f32, tag="proj", name="warmup")
        for _ in range(20):
            nc.tensor.matmul(wps, lhsT=warm_t[:, 0:128], rhs=warm_t[:, 128:640],
                             start=True, stop=True)
        emit_qkT(wk_sb, kT, 0, 0)
        emit_qkT(wq_sb, qT, 0, 0)

        # ---- normalize helper (broadcast 1/denom from DRAM) ----
        a_sb_store = {}
        rcd_store = {}

        def normalize(qc, p2, rcd):
            # multiply a_sb values by broadcast 1/denominator rows of
            # rcd (DRAM [64,64] f32; head h's 512 recips at flat offset h*512),
            # write the normalized halves into attn_norm
            qsl = slice(qc * 512, (qc + 1) * 512)
            a2 = a_sb_store.pop((qc, p2))
            for half in (0, 1):
                src = rcd[8 * (2 * p2 + half):8 * (2 * p2 + half) + 1, :]
                bc_ap = bass.AP(tensor=src.tensor, offset=src.offset,
                                ap=[[0, 64], [1, 512]])
                bc = bcastp.tile([64, 512], f32, tag="bc")
                nc.sync.dma_start(out=bc, in_=bc_ap)
                if half == 0:
                    nc.vector.tensor_mul(
                        attn_norm[p2][0:64, qsl], a2[0:64, 0:512], bc)
                else:
                    # DVE supports lane-indexed partition offsets: write the
                    # odd head's rows straight to partitions 64..127
                    nc.vector.tensor_mul(
                        attn_norm[p2][64:128, qsl], a2[0:64, 512:1024], bc)

        def normalize_q3(p2, rc1):
            # qc3 variant: broadcast the reciprocals across partitions with
            # two K=1 PE matmuls (ones outer product) instead of the DRAM
            # round trip -- the tail chain this gates loses ~2 DMA hops and
            # the PE stays warm through it.
            qsl = slice(1536, 2048)
            a2 = a_sb_store.pop((NQC - 1, p2))
            # ONE proj tile for both halves (col tiles at partitions 0/64) so
            # the pool keeps a free buffer for concurrent out-proj fillers
            bcp = proj_ps.tile([128, 512], f32, tag="proj", name=f"bcq3_{p2}")
            for half in (0, 1):
                nc.tensor.matmul(
                    bcp[64 * half:64 * half + 64, :], lhsT=ones_t[:, 0:64],
                    rhs=rc1[0:1, 512 * half:512 * half + 512],
                    start=True, stop=True,
                )
            for half in (0, 1):
                nc.vector.tensor_mul(
                    attn_norm[p2][64 * half:64 * half + 64, qsl],
                    a2[0:64, 512 * half:512 * half + 512],
                    bcp[64 * half:64 * half + 64, :])

        # ---- filler work queue: (due_iter, seq, cost_ns, eager, emit_fn) ----
        # due_iter = last iteration index at whose filler slot the unit may be
        # emitted and still precede (in program order) its first consumer.
        # eager = first iteration at which early emission is safe (data ready).
        fillers = []
        seq = [0]

        def add(due, cost, fn, eager=0):
            fillers.append((due, seq[0], cost, eager, fn))
            seq[0] += 1

        # staggered vhat memsets (GpSimd, no PE cost) ahead of their fills
        for st in range(2, NST):
            add(st - 5, 0, lambda s=st: nc.gpsimd.memset(vhat[s], 1.0),
                eager=st - 5)
        # stage-1: vhat tiles, due just before their consuming attnV
        for kt in range(NST):
            add(kt - 2, 1900, lambda k=kt: emit_vhat(k))
        # kT[0] chunks 1-3 feed scores iterations 4c..4c+3 (emitted 1 ahead)
        for c in range(1, NSC):
            add(4 * c - 2, 1900, lambda cc=c: emit_qkT(wk_sb, kT, 0, cc))
        # kT[p>=1]: all 4 chunks due before iteration 16p
        for p in range(1, NET):
            for c in range(NSC):
                add(16 * p + 4 * c - 6, 1900, lambda pp=p, cc=c: emit_qkT(wk_sb, kT, pp, cc))
        # qT[p] chunk qc due before iteration 64qc+16p
        for p in range(NET):
            for c in range(NQC):
                if (p, c) == (0, 0):
                    continue
                add(64 * c + 16 * p - 8, 1900, lambda pp=p, cc=c: emit_qkT(wq_sb, qT, pp, cc))
        # normalize muls of qc deferred past qc+1's p0 evacuation so the DVE
        # burst does not delay it (the evac gates qc+1 p1's attnV start)
        for qc in range(NQC - 1):
            for p2 in range(NET):
                due = 64 * (qc + 1) + 16 + p2
                add(due, 1400,
                    lambda q=qc, pp=p2: normalize(q, pp, rcd_store[q]),
                    eager=due)
        # qc3 per-p normalize chains (p0..p2) deferred off the boundaries
        dd_store = {}

        def qc3_norm(p2):
            dd = dd_store.pop(p2)
            rc1 = rcp.tile([1, 1024], f32, tag="rc2", name=f"rcq3_{p2}")
            nc.vector.reciprocal_approx_fast(out=rc1, in_=dd)
            normalize_q3(p2, rc1)

        for p2 in range(NET - 1):
            due = 64 * (NQC - 1) + 16 * p2 + 18
            add(due, 1400, lambda pp=p2: qc3_norm(pp), eager=due)
        # out-projection of qc during qc+1's pass (whole units; eager=due so
        # a unit is never emitted before its attn_norm inputs exist)
        for qc in range(NQC - 1):
            for i, (st, dc) in enumerate(
                    [(st, dc) for st in range(4 * qc, 4 * qc + 4) for dc in range(2)]):
                due = 64 * (qc + 1) + 22 + 3 * i
                add(due, 1100, lambda s=st, dd=dc: emit_outproj(s, dd), eager=due)
        tail_units = [(st, dc) for st in range(4 * (NQC - 1), 4 * NQC) for dc in range(2)]
        fillers.sort(key=lambda t: (t[0], t[1]))
        fq = list(fillers)

        SLACK = 330.0  # eager filler budget per iteration (ns of PE time)

        # ---- attention stream ----
        stream = [(qc, p, kt) for qc in range(NQC) for p in range(NET) for kt in range(NST)]
        sc_tiles = {}

        def emit_sc(i):
            qc, p, kt = stream[i]
            t = scores_ps.tile([128, 1024], f32, tag="scores")
            sc_tiles[i] = t
            nc.tensor.matmul(
                t[:, 0:512],
                lhsT=kT[p][0:HD, kt * 128:(kt + 1) * 128],
                rhs=qT[p][0:HD, qc * 512:(qc + 1) * 512],
                start=True, stop=True,
            )
            nc.tensor.matmul(
                t[:, 512:1024],
                lhsT=kT[p][64:64 + HD, kt * 128:(kt + 1) * 128],
                rhs=qT[p][64:64 + HD, qc * 512:(qc + 1) * 512],
                start=True, stop=True,
            )

        def dup_sc_half(i):
            # redundant re-emit of one scores matmul (same psum, same result):
            # pure PE-warmth padding so the HAM clock gate never sees an idle
            # MID window (at <70% busy the PE is held at 1.2 GHz half the run).
            # Always row group 0 so consecutive dups serialize (busy time).
            qc, p, kt = stream[i]
            t = sc_tiles[i]
            nc.tensor.matmul(
                t[:, 0:512],
                lhsT=kT[p][0:HD, kt * 128:(kt + 1) * 128],
                rhs=qT[p][0:HD, qc * 512:(qc + 1) * 512],
                start=True, stop=True,
            )

        PAD_TARGET = 900.0  # pad only near-empty slots (stream base is 770)

        emit_sc(0)
        av_t = None
        budget = 0.0
        for i, (qc, p, kt) in enumerate(stream):
            if i + 1 < NIT:
                emit_sc(i + 1)
            # exp on ACT (PSUM -> SBUF bf16), scale folds the 1/sqrt(hd)
            ex = expp.tile([128, 1024], bf16, tag="exp")
            nc.scalar.activation(out=ex, in_=sc_tiles.pop(i), func=FT.Exp, scale=0.125)
            # ---- filler emission (before attnV: fills the PE's exp-wait).
            # Boundary slots (kt 15/0) stay filler-free on the eager path so
            # the av evacuation reaches the DVE queue head and the next
            # pass's first attnV is not stalled on the psum buffer.
            budget += SLACK
            emitted = 770.0  # scores pair + attnV pair
            at_boundary = kt >= NST - 1 or kt == 0
            while fq and (fq[0][0] <= i or (not at_boundary
                                            and budget >= fq[0][2] and fq[0][3] <= i)):
                due, _, cost, eager, fn = fq.pop(0)
                fn()
                budget -= cost
                emitted += cost
            if budget > 4 * SLACK:
                budget = 4 * SLACK
            # ---- PE-warmth padding (runs inside the PE's exp-wait): only
            # when the slot is otherwise PE-idle enough to risk a HAM
            # re-throttle window ----
            if i + 1 < NIT:
                while emitted + 218.0 <= PAD_TARGET:
                    dup_sc_half(i + 1)
                    emitted += 218.0
            # attnV accumulation for the head pair
            if kt == 0:
                av_t = attn_ps.tile([VW, 1024], f32, tag="attn", name=f"av{qc}_{p}")
            nc.tensor.matmul(
                av_t[:, 0:512], lhsT=vhat[kt][:, 2 * p, :], rhs=ex[:, 0:512],
                start=(kt == 0), stop=(kt == NST - 1), skip_group_check=True,
            )
            nc.tensor.matmul(
                av_t[:, 512:1024], lhsT=vhat[kt][:, 2 * p + 1, :], rhs=ex[:, 512:1024],
                start=(kt == 0), stop=(kt == NST - 1), skip_group_check=True,
            )
            if kt == NST - 1:
                # ---- (qc, p) boundary: evacuate + denominator gather ----
                a_sb = asbp.tile([VW, 1024], f32, tag="asb", name=f"asb{qc}_{p}")
                if (qc, p) == (NQC - 1, NET - 1):
                    # final pass gates the whole tail: copy the den row out
                    # first so the reciprocal chain overlaps the big evac
                    dd = rcp.tile([1, 1024], f32, tag="dd", name="dd3")
                    nc.vector.tensor_copy(out=dd, in_=av_t[64:65, :])
                    nc.vector.tensor_copy(out=a_sb, in_=av_t)
                    a_sb_store[(qc, p)] = a_sb
                    rc1 = rcp.tile([1, 1024], f32, tag="rc2")
                    nc.vector.reciprocal_approx_fast(out=rc1, in_=dd)
                    normalize_q3(p, rc1)
                    continue
                nc.vector.tensor_copy(out=a_sb, in_=av_t)
                a_sb_store[(qc, p)] = a_sb
                if qc < NQC - 1:
                    # head h's 512 denominators land at rows 8h..8h+7
                    nc.sync.dma_start(out=den[qc][16 * p:16 * p + 8, :],
                                      in_=a_sb[64:65, 0:512])
                    nc.sync.dma_start(out=den[qc][16 * p + 8:16 * p + 16, :],
                                      in_=a_sb[64:65, 512:1024])
                    if p == NET - 1:
                        # batched fast reciprocal for all 8 heads of this
                        # q-chunk; normalize muls run later as fillers
                        rc = rcp.tile([64, 64], f32, tag="rc")
                        nc.vector.reciprocal_approx_fast(out=rc, in_=den[qc])
                        rcd = dramp.tile([64, 64], f32, tag="rcd", name=f"rcd{qc}")
                        nc.sync.dma_start(out=rcd, in_=rc)
                        rcd_store[qc] = rcd
                else:
                    # last q-chunk: per-p den-row gather (DVE partition shift
                    # 64->0); reciprocal+normalize run as qc3_norm fillers
                    dd = rcp.tile([1, 1024], f32, tag="dd", name=f"dd{p}")
                    nc.vector.tensor_copy(out=dd, in_=a_sb[64:65, :])
                    dd_store[p] = dd

        # drain remaining fillers, then the last q-chunk's out-projection.
        # Emission order keeps all p3-independent matmuls ahead of the first
        # p3-gated one (in-order PE): six tiles (2 proj bufs + 4 bank-aligned
        # halves of the idle scores pool) carry p0..p2 partials while the
        # final normalize chain lands, then the p3 matmuls and the last two
        # full units drain.
        for due, _, cost, eager, fn in fq:
            fn()
        tail_ps = {}
        for j in (0, 1):
            st, dc = tail_units[j]
            tail_ps[j] = proj_ps.tile([128, 512], f32, tag="proj", name=f"tailop{j}")
            outproj_mms(tail_ps[j], st, dc, (0, 1, 2))
        for j in (2, 3, 4, 5):
            st, dc = tail_units[j]
            if j % 2 == 0:
                full = scores_ps.tile([128, 1024], f32, tag="scores", name=f"tailsc{j}")
                tail_ps[j] = full[:, 0:512]
                tail_ps[j + 1] = full[:, 512:1024]
            outproj_mms(tail_ps[j], st, dc, (0, 1, 2))
        # bridge the normalize-chain wait with dummies (into the freed attn
        # bank) so the HAM clock gate stays at 2.4 GHz for the p3 matmuls
        wp2 = attn_ps.tile([VW, 1024], f32, tag="attn", name="tailwarm")
        for _ in range(10):
            nc.tensor.matmul(wp2[0:64, 0:512], lhsT=warm_t[:, 0:64],
                             rhs=warm_t[:, 128:640], start=True, stop=True)
        for j in range(6):
            st, dc = tail_units[j]
            ps = tail_ps.pop(j)
            outproj_mms(ps, st, dc, (3,))
            outproj_finish(ps, st, dc, engine=("scalar" if j % 2 else None))
        for j, (st, dc) in enumerate(tail_units[6:]):
            ps = proj_ps.tile([128, 512], f32, tag="proj", name=f"tailop2_{j}")
            outproj_mms(ps, st, dc, range(NET))
            outproj_finish(ps, st, dc, engine=("scalar" if j % 2 else None))

    if do_compile:
        nc.compile()
    return nc


_NC_CACHE = {}


def _get_nc():
    if "nc" not in _NC_CACHE:
        _NC_CACHE["nc"] = build_nc()
    return _NC_CACHE["nc"]


def _ptile(a):
    """[T*128, F] -> [128, T, F] (partition-major device layout)."""
    t = a.shape[0] // 128
    return np.ascontiguousarray(
        a.reshape(t, 128, a.shape[1]).transpose(1, 0, 2))


def shard_inputs(x, w_qkv, w_out):
    """Host-side shard + layout prep. Returns in_maps for 8 cores."""
    D = D_FULL
    E = HEADS_PER_CORE * HD
    in_maps = []
    for core in range(N_CORES):
        b, g = core // 2, core % 2
        cs = slice(g * E, (g + 1) * E)
        in_maps.append({
            "xT": _ptile(np.ascontiguousarray(x[b].T).astype(BF16)),
            "wq": _ptile(w_qkv[:, 0 * D:1 * D][:, cs].astype(BF16)),
            "wk": _ptile(w_qkv[:, 1 * D:2 * D][:, cs].astype(BF16)),
            "wv": _ptile(w_qkv[:, 2 * D:3 * D][:, cs].astype(BF16)),
            "wo": _ptile(w_out[cs, :].astype(BF16)),
        })
    return in_maps


def kernel(x, w_qkv, w_out):
    from concourse.bass_utils import run_bass_kernel_spmd

    x = np.asarray(x)
    w_qkv = np.asarray(w_qkv)
    w_out = np.asarray(w_out)
    nc = _get_nc()
    in_maps = shard_inputs(x, w_qkv, w_out)
    res = run_bass_kernel_spmd(nc, in_maps, list(range(N_CORES)))
    outs = [res.results[i]["out"] for i in range(N_CORES)]
    full = np.empty((B_FULL, S_FULL, D_FULL), np.float32)
    for b in range(B_FULL):
        full[b] = outs[2 * b] + outs[2 * b + 1]
    return full


# revision 44
# speedup vs baseline: 1.0157x; 1.0157x over previous
"""Trainium2 Bass kernel for nn_AttentionModel_88905823027207 (v5).

Full inputs:  x [4, 2048, 1024] f32, w_qkv [1024, 3072] f32, w_out [1024, 1024] f32
Full output:  [4, 2048, 1024] f32  (multi-head attention, 16 heads, + out proj)

Sharding: 8 cores = (batch b in 0..3) x (head-group g in 0..1).
Each core computes 8 heads of one batch element and the partial out-projection
for its head-group's rows of w_out; the host sums the two partials per batch.

v5 structure: the kernel streams 256 attention iterations, each gated by one
~1.11us [128,1024] exp ACTIVATE (the ACT roofline, ~285us total).  All bf16:
fp8 was measured to be numerically unusable here -- attention output is a
weighted MEAN of v, so the signal shrinks exactly as fast as averaged
quantization noise and every fp8 tensor (x/w/v/exp) contributes its full
~1.8-4% elementwise error to the output (measured 6.5% total vs the 2e-2
budget).  Relative to the v2 baseline the wins are pure scheduling:
  - stage-1 (qkv projections) and the out-projection run as due-date fillers
    inside the stream instead of a 76us solo-PE prefix and a 36us tail;
  - redundant scores matmuls pad PE-idle slots so the HAM clock gate never
    sees an idle MID window (at <70% busy the PE is held at 1.2 GHz);
  - input DMAs are ordered so iteration 0 unblocks after two transfers;
  - denominator reciprocals run 64 lanes wide ([8,512]->[64,64] reshape) via
    the fast approximate reciprocal (~8x+5x cheaper);
  - the final normalize chain DMA-shifts a_sb halves and broadcasts recips
    across all 128 partitions so one mul writes attn_norm directly, and the
    last pass copies its denominator row out of PSUM before the big
    evacuation; tail evacuations alternate between the (idle) ACT and DVE.
"""

import numpy as np
import ml_dtypes

BF16 = ml_dtypes.bfloat16

# Full-problem dims (hardcoded per harness contract)
B_FULL, S_FULL, D_FULL, H_FULL, HD = 4, 2048, 1024, 16, 64
N_CORES = 8
HEADS_PER_CORE = H_FULL // 2  # 8


def build_nc(S=2048, D=1024, heads=8, debug=False, do_compile=True):
    """Build + compile the per-core Bass program."""
    import concourse.bass as bass
    import concourse.mybir as mybir
    import concourse.tile as tile
    from concourse import bacc

    f32 = mybir.dt.float32
    bf16 = mybir.dt.bfloat16
    FT = mybir.ActivationFunctionType

    E = heads * HD              # per-core head channels (512)
    NDT = D // 128              # d-tiles (8)
    NST = S // 128              # s-tiles / k-tiles (16)
    NSC = S // 512              # 512-wide s-chunks (4)
    NET = E // 128              # e-tiles == head pairs (4)
    NQC = S // 512              # q-chunks (4)
    VW = 65                     # v columns per head incl. ones column
    NIT = NQC * NET * NST       # 256 attention iterations

    nc = bacc.Bacc("TRN2", target_bir_lowering=False, debug=debug)

    # inputs are host-prearranged to [partition, tile, free] so every input
    # DMA is a regular low-descriptor pattern (strided-gather issue cost on
    # the sync queue was ~1-3us per DMA and serialized the whole startup)
    # xT is chunk-major in DRAM so each s-chunk DMA reads one fully
    # contiguous 8KB run per partition (strided reads measured ~220 GB/s
    # vs ~358 peak, and the first chunk gates the whole pipeline start)
    xT_d = nc.dram_tensor("xT", [128, NSC, NDT, 512], bf16, kind="ExternalInput")
    wq_d = nc.dram_tensor("wq", [128, NDT, E], bf16, kind="ExternalInput")
    wk_d = nc.dram_tensor("wk", [128, NDT, E], bf16, kind="ExternalInput")
    wv_d = nc.dram_tensor("wv", [128, NDT, E], bf16, kind="ExternalInput")
    wo_d = nc.dram_tensor("wo", [128, NET, D], bf16, kind="ExternalInput")
    out_d = nc.dram_tensor("out", [S, D], f32, kind="ExternalOutput")

    from contextlib import ExitStack

    with tile.TileContext(nc) as tc, ExitStack() as ctx:
        const = ctx.enter_context(tc.tile_pool(name="const", bufs=1))
        proj_ps = ctx.enter_context(tc.tile_pool(name="proj_ps", bufs=2, space="PSUM"))
        scores_ps = ctx.enter_context(tc.tile_pool(name="scores_ps", bufs=2, space="PSUM"))
        attn_ps = ctx.enter_context(tc.tile_pool(name="attn_ps", bufs=1, space="PSUM"))
        # deep exp pool: attnV may lag up to 9 iterations behind the ACT
        # stream, letting the PE absorb the front-loaded stage-1 burst
        # (vhat/kT deadlines) without stalling the ACT engine
        expp = ctx.enter_context(tc.tile_pool(name="expp", bufs=8))
        asbp = ctx.enter_context(tc.tile_pool(name="asbp", bufs=5))
        rcp = ctx.enter_context(tc.tile_pool(name="rcp", bufs=2))
        bcastp = ctx.enter_context(tc.tile_pool(name="bcastp", bufs=4))
        outst = ctx.enter_context(tc.tile_pool(name="outst", bufs=3))
        dramp = ctx.enter_context(tc.tile_pool(name="dramp", bufs=4, space="DRAM"))

        # ---- persistent SBUF tensors ----
        xT_sb = const.tile([128, NDT, S], bf16, tag="xT_sb")
        wq_sb = const.tile([128, NDT, E], bf16, tag="wq_sb")
        wk_sb = const.tile([128, NDT, E], bf16, tag="wk_sb")
        wv_sb = const.tile([128, NDT, E], bf16, tag="wv_sb")
        wo_sb = const.tile([128, NET, D], bf16, tag="wo_sb")
        qT = [const.tile([128, S], bf16, tag=f"qT{p}", name=f"qT{p}") for p in range(NET)]
        kT = [const.tile([128, S], bf16, tag=f"kT{p}", name=f"kT{p}") for p in range(NET)]
        vhat = [const.tile([128, heads, VW], bf16, tag=f"vh{st}", name=f"vh{st}") for st in range(NST)]
        attn_norm = [const.tile([128, S], bf16, tag=f"an{p}", name=f"an{p}") for p in range(NET)]
        # denominators gathered as [64,64] (8 rows of 64 per head) so the
        # reciprocal runs 64 lanes wide instead of 8
        den = [const.tile([64, 64], f32, tag=f"den{qc}", name=f"den{qc}") for qc in range(NQC - 1)]

        # ---- input DMAs in first-use order (whole tensors: per-slice DMAs
        # measured slower -- issue serialization beats the pipelining win) ----
        nc.sync.dma_start(out=wk_sb, in_=wk_d.ap())
        for c in range(NSC):
            sl = slice(c * 512, (c + 1) * 512)
            nc.sync.dma_start(out=xT_sb[:, :, sl], in_=xT_d.ap()[:, c])
            if c == 0:
                nc.sync.dma_start(out=wq_sb, in_=wq_d.ap())
                nc.sync.dma_start(out=wv_sb, in_=wv_d.ap())
        nc.sync.dma_start(out=wo_sb, in_=wo_d.ap())

        # ---- emit helpers (each emits one atomic filler unit) ----
        def emit_vhat(st):
            # vhat[st][:, :, 0:64] = x[st-tile] @ wv (all heads, N=512)
            ps = proj_ps.tile([128, 512], f32, tag="proj")
            for dt in range(NDT):
                nc.tensor.matmul(
                    ps,
                    lhsT=xT_sb[:, dt, st * 128:(st + 1) * 128],
                    rhs=wv_sb[:, dt, :],
                    start=(dt == 0),
                    stop=(dt == NDT - 1),
                )
            nc.vector.tensor_copy(
                out=vhat[st][:, :, 0:HD],
                in_=ps.rearrange("q (h c) -> q h c", c=HD),
            )

        def emit_qkT(w_sb, dstT, p, c):
            # dstT[p][:, chunk c] = w[:, pair p].T @ x[:, chunk c] (N=512)
            ps = proj_ps.tile([128, 512], f32, tag="proj")
            for dt in range(NDT):
                nc.tensor.matmul(
                    ps,
                    lhsT=w_sb[:, dt, p * 128:(p + 1) * 128],
                    rhs=xT_sb[:, dt, c * 512:(c + 1) * 512],
                    start=(dt == 0),
                    stop=(dt == NDT - 1),
                )
            nc.vector.tensor_copy(out=dstT[p][:, c * 512:(c + 1) * 512], in_=ps)

        def outproj_mms(ps, st, dc, plist):
            for p in plist:
                nc.tensor.matmul(
                    ps,
                    lhsT=attn_norm[p][:, st * 128:(st + 1) * 128],
                    rhs=wo_sb[:, p, dc * 512:(dc + 1) * 512],
                    start=(p == 0),
                    stop=(p == NET - 1),
                )

        def outproj_finish(ps, st, dc, engine=None):
            ot = outst.tile([128, 512], f32, tag="ot")
            out_ap = out_d.ap()[st * 128:(st + 1) * 128, dc * 512:(dc + 1) * 512]
            if engine == "scalar":
                # tail only: ACT is idle after the last exp; splitting the
                # evacuations and the final stores across the two hwdge
                # queues halves the end-of-kernel drain
                nc.scalar.copy(out=ot, in_=ps)
                nc.scalar.dma_start(out=out_ap, in_=ot)
            else:
                nc.vector.tensor_copy(out=ot, in_=ps)
                nc.sync.dma_start(out=out_ap, in_=ot)

        def emit_outproj(st, dc):
            ps = proj_ps.tile([128, 512], f32, tag="proj")
            outproj_mms(ps, st, dc, range(NET))
            outproj_finish(ps, st, dc)

        # ---- prefix: first vhat memsets (ones columns) on the idle GpSimd
        # engine (keeps the DVE free for the first evacuations; the rest are
        # staggered fillers) + the two stage-1 units gating iteration 0's
        # scores: kT[0] c0, qT[0] c0.  vhat kt0/kt1 run as due<0 fillers
        # inside the stream, under ACT(0).
        nc.gpsimd.memset(vhat[0], 1.0)
        nc.gpsimd.memset(vhat[1], 1.0)
        # constants for the tail's PE-broadcast normalize
        ones_t = const.tile([1, 128], f32, tag="ones_t")
        nc.gpsimd.memset(ones_t, 1.0)
        # PE warmup: the HAM clock gate releases 2.4 GHz only after ~3.4us of
        # sustained activity; a handful of dummy matmuls during the DMA wait
        # means kT[0]c0/qT[0]c0 and the early stream run warm
        warm_t = const.tile([128, 640], bf16, tag="warm_t")
        nc.gpsimd.memset(warm_t, 0.0)
        # ~28 dummies span the ~10us input-DMA wait: HAM warms ~3.4us in and
        # kT[0]c0 plus the early stream then run at 2.4 GHz from the start
        wps = proj_ps.tile([128, 512], f32, tag="proj", name="warmup")
        for _ in range(20):
            nc.tensor.matmul(wps, lhsT=warm_t[:, 0:128], rhs=warm_t[:, 128:640],
                             start=True, stop=True)
        emit_qkT(wk_sb, kT, 0, 0)
        emit_qkT(wq_sb, qT, 0, 0)

        # ---- normalize helper (broadcast 1/denom from DRAM) ----
        a_sb_store = {}
        rcd_store = {}

        def normalize(qc, p2, rcd):
            # multiply a_sb values by broadcast 1/denominator rows of
            # rcd (DRAM [64,64] f32; head h's 512 recips at flat offset h*512),
            # write the normalized halves into attn_norm
            qsl = slice(qc * 512, (qc + 1) * 512)
            a2 = a_sb_store.pop((qc, p2))
            for half in (0, 1):
                src = rcd[8 * (2 * p2 + half):8 * (2 * p2 + half) + 1, :]
                bc_ap = bass.AP(tensor=src.tensor, offset=src.offset,
                                ap=[[0, 64], [1, 512]])
                bc = bcastp.tile([64, 512], f32, tag="bc")
                nc.sync.dma_start(out=bc, in_=bc_ap)
                if half == 0:
                    nc.vector.tensor_mul(
                        attn_norm[p2][0:64, qsl], a2[0:64, 0:512], bc)
                else:
                    # DVE supports lane-indexed partition offsets: write the
                    # odd head's rows straight to partitions 64..127
                    nc.vector.tensor_mul(
                        attn_norm[p2][64:128, qsl], a2[0:64, 512:1024], bc)

        def normalize_q3(p2, rc1):
            # qc3 variant: broadcast the reciprocals across partitions with
            # two K=1 PE matmuls (ones outer product) instead of the DRAM
            # round trip -- the tail chain this gates loses ~2 DMA hops and
            # the PE stays warm through it.
            qsl = slice(1536, 2048)
            a2 = a_sb_store.pop((NQC - 1, p2))
            # ONE proj tile for both halves (col tiles at partitions 0/64) so
            # the pool keeps a free buffer for concurrent out-proj fillers
            bcp = proj_ps.tile([128, 512], f32, tag="proj", name=f"bcq3_{p2}")
            for half in (0, 1):
                nc.tensor.matmul(
                    bcp[64 * half:64 * half + 64, :], lhsT=ones_t[:, 0:64],
                    rhs=rc1[0:1, 512 * half:512 * half + 512],
                    start=True, stop=True,
                )
            for half in (0, 1):
                nc.vector.tensor_mul(
                    attn_norm[p2][64 * half:64 * half + 64, qsl],
                    a2[0:64, 512 * half:512 * half + 512],
                    bcp[64 * half:64 * half + 64, :])

        # ---- filler work queue: (due_iter, seq, cost_ns, eager, emit_fn) ----
        # due_iter = last iteration index at whose filler slot the unit may be
        # emitted and still precede (in program order) its first consumer.
        # eager = first iteration at which early emission is safe (data ready).
        fillers = []
        seq = [0]

        def add(due, cost, fn, eager=0):
            fillers.append((due, seq[0], cost, eager, fn))
            seq[0] += 1

        # staggered vhat memsets (GpSimd, no PE cost) ahead of their fills
        for st in range(2, NST):
            add(st - 5, 0, lambda s=st: nc.gpsimd.memset(vhat[s], 1.0),
                eager=st - 5)
        # stage-1: vhat tiles, due just before their consuming attnV
        for kt in range(NST):
            add(kt - 2, 1900, lambda k=kt: emit_vhat(k))
        # kT[0] chunks 1-3 feed scores iterations 4c..4c+3 (emitted 1 ahead)
        for c in range(1, NSC):
            add(4 * c - 2, 1900, lambda cc=c: emit_qkT(wk_sb, kT, 0, cc))
        # kT[p>=1]: all 4 chunks due before iteration 16p
        for p in range(1, NET):
            for c in range(NSC):
                add(16 * p + 4 * c - 6, 1900, lambda pp=p, cc=c: emit_qkT(wk_sb, kT, pp, cc))
        # qT[p] chunk qc due before iteration 64qc+16p
        for p in range(NET):
            for c in range(NQC):
                if (p, c) == (0, 0):
                    continue
                add(64 * c + 16 * p - 8, 1900, lambda pp=p, cc=c: emit_qkT(wq_sb, qT, pp, cc))
        # normalize muls of qc deferred past qc+1's p0 evacuation so the DVE
        # burst does not delay it (the evac gates qc+1 p1's attnV start)
        for qc in range(NQC - 1):
            for p2 in range(NET):
                due = 64 * (qc + 1) + 16 + p2
                add(due, 1400,
                    lambda q=qc, pp=p2: normalize(q, pp, rcd_store[q]),
                    eager=due)
        # qc3 per-p normalize chains (p0..p2) deferred off the boundaries
        dd_store = {}

        def qc3_norm(p2):
            dd = dd_store.pop(p2)
            rc1 = rcp.tile([1, 1024], f32, tag="rc2", name=f"rcq3_{p2}")
            nc.vector.reciprocal_approx_fast(out=rc1, in_=dd)
            normalize_q3(p2, rc1)

        for p2 in range(NET - 1):
            due = 64 * (NQC - 1) + 16 * p2 + 18
            add(due, 1400, lambda pp=p2: qc3_norm(pp), eager=due)
        # out-projection of qc during qc+1's pass (whole units; eager=due so
        # a unit is never emitted before its attn_norm inputs exist)
        for qc in range(NQC - 1):
            for i, (st, dc) in enumerate(
                    [(st, dc) for st in range(4 * qc, 4 * qc + 4) for dc in range(2)]):
                due = 64 * (qc + 1) + 22 + 3 * i
                add(due, 1100, lambda s=st, dd=dc: emit_outproj(s, dd), eager=due)
        tail_units = [(st, dc) for st in range(4 * (NQC - 1), 4 * NQC) for dc in range(2)]
        fillers.sort(key=lambda t: (t[0], t[1]))
        fq = list(fillers)

        SLACK = 330.0  # eager filler budget per iteration (ns of PE time)

        # ---- attention stream ----
        stream = [(qc, p, kt) for qc in range(NQC) for p in range(NET) for kt in range(NST)]
        sc_tiles = {}

        def emit_sc(i):
            qc, p, kt = stream[i]
            t = scores_ps.tile([128, 1024], f32, tag="scores")
            sc_tiles[i] = t
            nc.tensor.matmul(
                t[:, 0:512],
                lhsT=kT[p][0:HD, kt * 128:(kt + 1) * 128],
                rhs=qT[p][0:HD, qc * 512:(qc + 1) * 512],
                start=True, stop=True,
            )
            nc.tensor.matmul(
                t[:, 512:1024],
                lhsT=kT[p][64:64 + HD, kt * 128:(kt + 1) * 128],
                rhs=qT[p][64:64 + HD, qc * 512:(qc + 1) * 512],
                start=True, stop=True,
            )

        def dup_sc_half(i):
            # redundant re-emit of one scores matmul (same psum, same result):
            # pure PE-warmth padding so the HAM clock gate never sees an idle
            # MID window (at <70% busy the PE is held at 1.2 GHz half the run).
            # Always row group 0 so consecutive dups serialize (busy time).
            qc, p, kt = stream[i]
            t = sc_tiles[i]
            nc.tensor.matmul(
                t[:, 0:512],
                lhsT=kT[p][0:HD, kt * 128:(kt + 1) * 128],
                rhs=qT[p][0:HD, qc * 512:(qc + 1) * 512],
                start=True, stop=True,
            )

        PAD_TARGET = 900.0  # pad only near-empty slots (stream base is 770)

        emit_sc(0)
        av_t = None
        budget = 0.0
        for i, (qc, p, kt) in enumerate(stream):
            if i + 1 < NIT:
                emit_sc(i + 1)
            # exp on ACT (PSUM -> SBUF bf16), scale folds the 1/sqrt(hd)
            ex = expp.tile([128, 1024], bf16, tag="exp")
            nc.scalar.activation(out=ex, in_=sc_tiles.pop(i), func=FT.Exp, scale=0.125)
            # ---- filler emission (before attnV: fills the PE's exp-wait).
            # Boundary slots (kt 15/0) stay filler-free on the eager path so
            # the av evacuation reaches the DVE queue head and the next
            # pass's first attnV is not stalled on the psum buffer.
            budget += SLACK
            emitted = 770.0  # scores pair + attnV pair
            at_boundary = kt >= NST - 1 or kt == 0
            while fq and (fq[0][0] <= i or (not at_boundary
                                            and budget >= fq[0][2] and fq[0][3] <= i)):
                due, _, cost, eager, fn = fq.pop(0)
                fn()
                budget -= cost
                emitted += cost
            if budget > 4 * SLACK:
                budget = 4 * SLACK
            # ---- PE-warmth padding (runs inside the PE's exp-wait): only
            # when the slot is otherwise PE-idle enough to risk a HAM
            # re-throttle window ----
            if i + 1 < NIT:
                while emitted + 218.0 <= PAD_TARGET:
                    dup_sc_half(i + 1)
                    emitted += 218.0
            # attnV accumulation for the head pair
            if kt == 0:
                av_t = attn_ps.tile([VW, 1024], f32, tag="attn", name=f"av{qc}_{p}")
            nc.tensor.matmul(
                av_t[:, 0:512], lhsT=vhat[kt][:, 2 * p, :], rhs=ex[:, 0:512],
                start=(kt == 0), stop=(kt == NST - 1), skip_group_check=True,
            )
            nc.tensor.matmul(
                av_t[:, 512:1024], lhsT=vhat[kt][:, 2 * p + 1, :], rhs=ex[:, 512:1024],
                start=(kt == 0), stop=(kt == NST - 1), skip_group_check=True,
            )
            if kt == NST - 1:
                # ---- (qc, p) boundary: evacuate + denominator gather ----
                a_sb = asbp.tile([VW, 1024], f32, tag="asb", name=f"asb{qc}_{p}")
                if (qc, p) == (NQC - 1, NET - 1):
                    # final pass gates the whole tail: copy the den row out
                    # first so the reciprocal chain overlaps the big evac
                    dd = rcp.tile([1, 1024], f32, tag="dd", name="dd3")
                    nc.vector.tensor_copy(out=dd, in_=av_t[64:65, :])
                    nc.vector.tensor_copy(out=a_sb, in_=av_t)
                    a_sb_store[(qc, p)] = a_sb
                    rc1 = rcp.tile([1, 1024], f32, tag="rc2")
                    nc.vector.reciprocal_approx_fast(out=rc1, in_=dd)
                    normalize_q3(p, rc1)
                    continue
                nc.vector.tensor_copy(out=a_sb, in_=av_t)
                a_sb_store[(qc, p)] = a_sb
                if qc < NQC - 1:
                    # head h's 512 denominators land at rows 8h..8h+7
                    nc.sync.dma_start(out=den[qc][16 * p:16 * p + 8, :],
                                      in_=a_sb[64:65, 0:512])
                    nc.sync.dma_start(out=den[qc][16 * p + 8:16 * p + 16, :],
                                      in_=a_sb[64:65, 512:1024])
                    if p == NET - 1:
                        # batched fast reciprocal for all 8 heads of this
                        # q-chunk; normalize muls run later as fillers
                        rc = rcp.tile([64, 64], f32, tag="rc")
                        nc.vector.reciprocal_approx_fast(out=rc, in_=den[qc])
                        rcd = dramp.tile([64, 64], f32, tag="rcd", name=f"rcd{qc}")
                        nc.sync.dma_start(out=rcd, in_=rc)
                        rcd_store[qc] = rcd
                else:
                    # last q-chunk: per-p den-row gather (DVE partition shift
                    # 64->0); reciprocal+normalize run as qc3_norm fillers
                    dd = rcp.tile([1, 1024], f32, tag="dd", name=f"dd{p}")
                    nc.vector.tensor_copy(out=dd, in_=a_sb[64:65, :])
                    dd_store[p] = dd

        # drain remaining fillers, then the last q-chunk's out-projection.
        # Emission order keeps all p3-independent matmuls ahead of the first
        # p3-gated one (in-order PE): six tiles (2 proj bufs + 4 bank-aligned
        # halves of the idle scores pool) carry p0..p2 partials while the
        # final normalize chain lands, then the p3 matmuls and the last two
        # full units drain.
        for due, _, cost, eager, fn in fq:
            fn()
        tail_ps = {}
        for j in (0, 1):
            st, dc = tail_units[j]
            tail_ps[j] = proj_ps.tile([128, 512], f32, tag="proj", name=f"tailop{j}")
            outproj_mms(tail_ps[j], st, dc, (0, 1, 2))
        for j in (2, 3, 4, 5):
            st, dc = tail_units[j]
            if j % 2 == 0:
                full = scores_ps.tile([128, 1024], f32, tag="scores", name=f"tailsc{j}")
                tail_ps[j] = full[:, 0:512]
                tail_ps[j + 1] = full[:, 512:1024]
            outproj_mms(tail_ps[j], st, dc, (0, 1, 2))
        for j in range(6):
            st, dc = tail_units[j]
            ps = tail_ps.pop(j)
            outproj_mms(ps, st, dc, (3,))
            outproj_finish(ps, st, dc, engine=("scalar" if j % 2 else None))
        for j, (st, dc) in enumerate(tail_units[6:]):
            ps = proj_ps.tile([128, 512], f32, tag="proj", name=f"tailop2_{j}")
            outproj_mms(ps, st, dc, range(NET))
            outproj_finish(ps, st, dc, engine=("scalar" if j % 2 else None))

    if do_compile:
        nc.compile()
    return nc


_NC_CACHE = {}


def _get_nc():
    if "nc" not in _NC_CACHE:
        _NC_CACHE["nc"] = build_nc()
    return _NC_CACHE["nc"]


def _ptile(a):
    """[T*128, F] -> [128, T, F] (partition-major device layout)."""
    t = a.shape[0] // 128
    return np.ascontiguousarray(
        a.reshape(t, 128, a.shape[1]).transpose(1, 0, 2))


def shard_inputs(x, w_qkv, w_out):
    """Host-side shard + layout prep. Returns in_maps for 8 cores."""
    D = D_FULL
    E = HEADS_PER_CORE * HD
    in_maps = []
    for core in range(N_CORES):
        b, g = core // 2, core % 2
        cs = slice(g * E, (g + 1) * E)
        xt = np.ascontiguousarray(x[b].T).astype(BF16)
        # [t*128+p, c*512+s] -> [p, c, t, s] (chunk-major per partition)
        xt = np.ascontiguousarray(
            xt.reshape(8, 128, 4, 512).transpose(1, 2, 0, 3))
        in_maps.append({
            "xT": xt,
            "wq": _ptile(w_qkv[:, 0 * D:1 * D][:, cs].astype(BF16)),
            "wk": _ptile(w_qkv[:, 1 * D:2 * D][:, cs].astype(BF16)),
            "wv": _ptile(w_qkv[:, 2 * D:3 * D][:, cs].astype(BF16)),
            "wo": _ptile(w_out[cs, :].astype(BF16)),
        })
    return in_maps


def kernel(x, w_qkv, w_out):
    from concourse.bass_utils import run_bass_kernel_spmd

    x = np.asarray(x)
    w_qkv = np.asarray(w_qkv)
    w_out = np.asarray(w_out)
    nc = _get_nc()
    in_maps = shard_inputs(x, w_qkv, w_out)
    res = run_bass_kernel_spmd(nc, in_maps, list(range(N_CORES)))
    outs = [res.results[i]["out"] for i in range(N_CORES)]
    full = np.empty((B_FULL, S_FULL, D_FULL), np.float32)
    for b in range(B_FULL):
        full[b] = outs[2 * b] + outs[2 * b + 1]
    return full
